# revision 8
# baseline (speedup 1.0000x reference)
"""Trainium2 Bass kernel for nn_FPSWE_40303973105696.

Computation (see problem reference): project X onto P directions, sort along
N, linearly interpolate N->M quantiles, subtract from ref, contract with
weight.

For these shapes (N=2048, M=1024) the quantile-interpolation gather is exactly
ind[m] = 2m, so the folded contraction weights are a simple even/odd
interleave of weight scaled by the interpolation fraction t[m]:

    out[b, p] = rw[p] - sum_n Xs[b, n, p] * W2[p, n]
    W2[p, 2m]   = weight[p, m] * (1 - t[m])
    W2[p, 2m+1] = weight[p, m] * t[m]
    rw[p]       = sum_m ref[m] * weight[p, m]

so W2 is built on-device from weight itself (no precomputed [P, N] upload).

Device kernel per core (data-parallel over B, core c handles batch c):
    1. transpose X[b] and theta_w via PE identity matmuls
    2. proj[p, n] = theta_w[p, :] @ X[b].T        (PE, bf16 -> fp32 PSUM)
    3. bitonic sort along n                       (DVE bf16, 66 stages)
    4. W2 rows from weight + t                    (GPSIMD, overlapped)
    5. acc[p] = sum_n Xs[p, n] * W2[p, n]         (GPSIMD mult + ACT accum)
    6. out[p] = rw[p] - acc[p]

All 8 row-chunks share ONE wide [128, 16384] tile pair, with columns in the
relabeled order of sortnet.py (col = 8*z(n) + chunk). The relabeling keeps
every compare-exchange operand's innermost contiguous run >= 8 unit-stride
elements, which is what the DVE's packed bf16 perf modes require; the
measured penalty for innermost runs of 1 (classic stride-1 bitonic stages)
is ~4x.
"""

import numpy as np

from concourse import bass, bacc, mybir
from concourse.ap import AP
from concourse.tile import TileContext

import sortnet

B, N, D, P, M = 8, 2048, 128, 1024, 1024
NT = P // 128          # 8 projection row-chunks of 128 partitions each
MM_CHUNK = 512         # matmul free-dim chunk (one PSUM bank)
N_CORES = 8

# sub-tile groups: (ci = chunks interleaved, global chunk ids, emu_frac =
# fraction of each std stage offloaded to the GPSIMD+ACT relu-emulated
# compare-exchange pipeline)
GROUPS_CFG = [
    (4, [0, 1, 2, 3], 0.0),
    (4, [4, 5, 6, 7], 0.0),
]

# debug knob: limit number of sort stages emitted (None = all)
STAGE_LIMIT = None
# benchmark knob: emit the whole kernel body this many times (timing only)
REPEAT = 1

FP = mybir.dt.float32
SD = mybir.dt.bfloat16  # sort dtype

_ALU = {"min": mybir.AluOpType.min, "max": mybir.AluOpType.max}

# input names that are per-core (sharded); everything else is replicated
_SHARDED = {"xnat"}

# packed replicated-input layout (bf16 elements): theta_w | weight |
# t-vector (replicated to 128 partitions) | identity | rw
O_THN = 0
O_WQ = O_THN + P * D
O_TV = O_WQ + P * M
O_EYE = O_TV + 128 * M
O_RW = O_EYE + 128 * 128
PACK_LEN = O_RW + 128 * NT


def _ap(tile_ap, dims, off):
    """Build an AP on a [128, W] tile from (col-dims, col-offset)."""
    base = tile_ap[:]
    w = base.ap[0][0]          # partition stride = row width
    return AP(base.tensor, int(base.offset) + off,
              [[w, 128]] + [[s, c] for s, c in dims])


def _build_kernel():
    nc = bacc.Bacc()

    xnat = nc.declare_dram_parameter("xnat", [N, D], SD, isOutput=False)   # X[b]
    pack = nc.declare_dram_parameter("pack", [1, PACK_LEN], SD, isOutput=False)
    out = nc.declare_dram_parameter("out", [128, NT], FP, isOutput=True)

    stages = sortnet.stages()
    if STAGE_LIMIT is not None:
        stages = stages[:STAGE_LIMIT]

    with TileContext(nc) as tc:
        with (
            tc.tile_pool(name="const", bufs=1) as const_pool,
            tc.tile_pool(name="xt", bufs=1) as xt_pool,
            tc.tile_pool(name="sa", bufs=1) as a_pool,
            tc.tile_pool(name="sb", bufs=1) as b_pool,
            tc.tile_pool(name="wq", bufs=2) as w_pool,
            tc.tile_pool(name="w2", bufs=1) as w2_pool,
            tc.tile_pool(name="prod", bufs=2) as prod_pool,
            tc.tile_pool(name="pst", bufs=2, space="PSUM") as pst_pool,
            tc.tile_pool(name="ps", bufs=2, space="PSUM") as psum_pool,
        ):
            eye_raw = const_pool.tile([128, 128], SD, tag="eyer")
            eye_sb = const_pool.tile([128, 128], SD, tag="eye")
            tv_sb = const_pool.tile([128, M], SD, tag="tv")
            tvb_sb = const_pool.tile([128, M], SD, tag="tvb")
            rw_sb = const_pool.tile([128, NT], SD, tag="rw")
            acc_sb = const_pool.tile([128, NT], FP, tag="acc")
            out_sb = const_pool.tile([128, NT], FP, tag="outsb")
            out_tmp = const_pool.tile([128, NT], FP, tag="outtmp")
            xn_sb = xt_pool.tile([128, N], SD, tag="xn")        # X[b] natural
            xt_sb = xt_pool.tile([D, N], SD, tag="xt")          # X[b].T
            thn_sb = xt_pool.tile([128, P], SD, tag="thn")      # theta_w natural
            tht_sb = xt_pool.tile([D, P], SD, tag="tht")        # theta_w.T

            nc.sync.dma_start(
                out=eye_raw[:],
                in_=pack[0:1, O_EYE:O_EYE + 128 * 128].rearrange(
                    "o (p q) -> (o p) q", q=128))
            nc.sync.dma_start(
                out=tv_sb[:],
                in_=pack[0:1, O_TV:O_TV + 128 * M].rearrange(
                    "o (p m) -> (o p) m", m=M))
            nc.sync.dma_start(
                out=rw_sb[:],
                in_=pack[0:1, O_RW:O_RW + 128 * NT].rearrange(
                    "o (p r) -> (o p) r", r=NT))
            # X[b] as [128, 16*128]: xn_sb[p, k*128+d] = X[k*128+p, d]
            nc.sync.dma_start(
                out=xn_sb.rearrange("p (k d) -> p k d", d=D),
                in_=xnat.rearrange("(k p) d -> p k d", p=128))
            # theta_w as [128, 8*128]: thn_sb[p, r*128+d] = theta_w[r*128+p, d]
            nc.sync.dma_start(
                out=thn_sb.rearrange("p (r d) -> p r d", d=D),
                in_=pack[0:1, O_THN:O_THN + P * D].rearrange(
                    "o (r p d) -> (o p) r d", p=128, d=D))
            # Bounce eye through ACT so Matmult (transpose) instructions never
            # carry two DMA-queue semaphore waits (walrus codegen limit).
            nc.scalar.copy(out=eye_sb[:], in_=eye_raw[:])
            # bounce tv through ACT so gpsimd W2-build ops carry at most one
            # DMA-queue semaphore wait
            nc.scalar.copy(out=tvb_sb[:], in_=tv_sb[:])

            # on-device transposes: X[b].T [D, N] and theta_w.T [D, P]
            for k in range(N // 128):
                ps = pst_pool.tile([128, 128], SD, tag="pst", name="pst")
                nc.tensor.transpose(
                    ps[:], xn_sb[:, k * 128:(k + 1) * 128], eye_sb[:])
                nc.scalar.copy(out=xt_sb[:, k * 128:(k + 1) * 128], in_=ps[:])
            for r in range(NT):
                ps = pst_pool.tile([128, 128], SD, tag="pst", name="pst")
                nc.tensor.transpose(
                    ps[:], thn_sb[:, r * 128:(r + 1) * 128], eye_sb[:])
                nc.scalar.copy(out=tht_sb[:, r * 128:(r + 1) * 128], in_=ps[:])

            # Per-group W2 (relabeled layout), built on GPSIMD
            ngr = len(GROUPS_CFG)
            w2_t, a_t, b_t = {}, {}, {}
            for g, (ci, gchunks, _ef) in enumerate(GROUPS_CFG):
                gw = ci * N
                w2_t[g] = w2_pool.tile([128, gw], FP, tag=f"w2f{g}", name=f"w2f{g}")
                a_t[g] = a_pool.tile([128, gw], SD, tag=f"aw{g}", name=f"aw{g}")
                b_t[g] = b_pool.tile([128, gw], SD, tag=f"bw{g}",
                                     name=f"bw{g}")
                for rl, gid in enumerate(gchunks):
                    w_sb = w_pool.tile([128, M], SD, tag="wq", name="wq")
                    nc.sync.dma_start(
                        out=w_sb[:],
                        in_=pack[0:1,
                                 O_WQ + gid * 128 * M:
                                 O_WQ + (gid + 1) * 128 * M].rearrange(
                            "o (p m) -> (o p) m", m=M))
                    # odd (e=1): W2 = w * t ; even (e=0): W2 = w - odd
                    for kind, od, oo, i0d, i0o, i1d, i1o in \
                            sortnet.w2_ops(rl, ci):
                        if kind == "mult":
                            nc.gpsimd.tensor_mul(_ap(w2_t[g], od, oo),
                                                 _ap(w_sb, i0d, i0o),
                                                 _ap(tvb_sb, i1d, i1o))
                        else:
                            nc.gpsimd.tensor_sub(_ap(w2_t[g], od, oo),
                                                 _ap(w_sb, i0d, i0o),
                                                 _ap(w2_t[g], i1d, i1o))

            # emu scratch (difference + relu tiles)
            d_t = a_pool.tile([128, 4096], SD, tag="demul", name="demul")
            r_t = a_pool.tile([128, 4096], SD, tag="remul", name="remul")

            def _dense(counts):
                dims = []
                prod = 1
                for c in reversed(counts):
                    dims.append((prod, c))
                    prod *= c
                dims.reverse()
                return dims

            def _emit_emu(src, dst, minop, maxop, emu_frac):
                """Split a std-stage op pair: DVE head slice + GPSIMD/ACT
                relu-emulated tail slice of dim 0."""
                _, od0, oo0, i0d, i0o, i1d, i1o = minop
                _, od1, oo1, _, _, _, _ = maxop
                s0, c0 = od0[0]
                k = int(round((1.0 - emu_frac) * c0))
                if s0 == 1:
                    k = max(0, min(c0, (k // 2) * 2))
                k = max(0, min(c0, k))

                def part(dims, off, lo, hi):
                    nd = [(dims[0][0], hi - lo)] + list(dims[1:])
                    return nd, off + dims[0][0] * lo

                if k > 0:
                    for alu, od, oo, j0d, j0o, j1d, j1o in (minop, maxop):
                        pd, po = part(od, oo, 0, k)
                        p0d, p0o = part(j0d, j0o, 0, k)
                        p1d, p1o = part(j1d, j1o, 0, k)
                        nc.vector.tensor_tensor(
                            _ap(dst, pd, po), _ap(src, p0d, p0o),
                            _ap(src, p1d, p1o), op=_ALU[alu])
                if k < c0:
                    lod, loo = part(i0d, i0o, k, c0)
                    hid, hio = part(i1d, i1o, k, c0)
                    om_d, om_o = part(od0, oo0, k, c0)
                    ox_d, ox_o = part(od1, oo1, k, c0)
                    counts = [c for _, c in lod]
                    dd = _dense(counts)
                    nc.gpsimd.tensor_sub(_ap(d_t, dd, 0),
                                         _ap(src, lod, loo),
                                         _ap(src, hid, hio))
                    nc.scalar.activation(_ap(r_t, dd, 0), _ap(d_t, dd, 0),
                                         mybir.ActivationFunctionType.Relu)
                    nc.gpsimd.tensor_sub(_ap(dst, om_d, om_o),
                                         _ap(src, lod, loo),
                                         _ap(r_t, dd, 0))
                    nc.gpsimd.tensor_add(_ap(dst, ox_d, ox_o),
                                         _ap(src, hid, hio),
                                         _ap(r_t, dd, 0))

            def emit_body(rep_i):
                # ---- phase A: projection matmuls, scattered into the
                # relabeled group layouts (group-major for overlap) ----
                for g, (ci, gchunks, _ef) in enumerate(GROUPS_CFG):
                    for rl, gid in enumerate(gchunks):
                        for c in range(N // MM_CHUNK):
                            ps = psum_pool.tile([128, MM_CHUNK], FP,
                                                tag="ps", name="ps")
                            nc.tensor.matmul(
                                ps[:],
                                lhsT=tht_sb[:, gid * 128:(gid + 1) * 128],
                                rhs=xt_sb[:, c * MM_CHUNK:(c + 1) * MM_CHUNK],
                                start=True, stop=True,
                            )
                            in_dims, out_dims, out_off = \
                                sortnet.proj_copy_aps(rl, c, ci)
                            nc.scalar.copy(
                                out=_ap(a_t[g], out_dims, out_off),
                                in_=_ap(ps, in_dims, 0))

                # ---- phase B: sort (groups round-robin per stage) ----
                curs = dict(a_t)
                oths = dict(b_t)
                for kind, val in stages:
                    for g, (ci, gchunks, ef) in enumerate(GROUPS_CFG):
                        ops = sortnet.stage_ops(kind, val, ci)
                        if ef > 0.0 and kind == "std" and len(ops) == 2:
                            _emit_emu(curs[g], oths[g], ops[0], ops[1], ef)
                            continue
                        for alu, od, oo, i0d, i0o, i1d, i1o in ops:
                            nc.vector.tensor_tensor(
                                _ap(oths[g], od, oo),
                                _ap(curs[g], i0d, i0o),
                                _ap(curs[g], i1d, i1o),
                                op=_ALU[alu])
                    curs, oths = oths, curs

                # ---- phase C: weighted reduction per row-chunk ----
                for g, (ci, gchunks, _ef) in enumerate(GROUPS_CFG):
                    for rl, gid in enumerate(gchunks):
                        rd, ro = sortnet.reduce_ap(rl, ci)
                        prod = prod_pool.tile([128, N], FP, tag="prod",
                                              name="prod")
                        nc.gpsimd.tensor_mul(prod[:],
                                             _ap(curs[g], rd, ro),
                                             _ap(w2_t[g], rd, ro))
                        nc.scalar.activation(
                            _ap(curs[g], rd, ro), prod[:],
                            mybir.ActivationFunctionType.Copy,
                            accum_out=acc_sb[:, gid:gid + 1])

                # accumulate across repeat bodies so none is dead code;
                # the final output is REPEAT * (rw - acc), divided on host
                if rep_i == 0:
                    nc.vector.tensor_sub(out_sb[:], rw_sb[:], acc_sb[:])
                else:
                    nc.vector.tensor_sub(out_tmp[:], rw_sb[:], acc_sb[:])
                    nc.vector.tensor_add(out_sb[:], out_sb[:], out_tmp[:])

            for _rep in range(REPEAT):
                emit_body(_rep)
            nc.sync.dma_start(out=out[:], in_=out_sb[:])

    return nc


_NC_CACHE = None


def _get_nc():
    global _NC_CACHE
    if _NC_CACHE is None:
        nc = _build_kernel()
        nc.finalize()   # Bacc: runs wait-splitting + register allocation
        _NC_CACHE = nc
    return _NC_CACHE


def _host_precompute(X, theta_w, ref, weight):
    """Global (all-core) input arrays, keyed by dram parameter name."""
    X = np.ascontiguousarray(np.asarray(X, dtype=np.float32))
    theta_w = np.ascontiguousarray(np.asarray(theta_w, dtype=np.float32))
    ref = np.asarray(ref, dtype=np.float32)
    weight = np.ascontiguousarray(np.asarray(weight, dtype=np.float32))

    x1d = np.linspace(0.0, 1.0, N + 2, dtype=np.float32)[1:-1]
    xnew = np.linspace(0.0, 1.0, M + 2, dtype=np.float32)[1:-1]
    ind = 2 * np.arange(M)      # == clip(searchsorted(x1d, xnew) - 1, 0, N-2)
    eps = np.float32(np.finfo(np.float32).eps)
    dx = x1d[1:] - x1d[:-1]
    t = ((xnew - x1d[ind]) / (eps + dx[ind])).astype(np.float32)

    rw = (weight.astype(np.float64) @ ref.astype(np.float64)).astype(np.float32)

    import ml_dtypes
    bf = ml_dtypes.bfloat16
    pack = np.empty(PACK_LEN, dtype=bf)
    pack[O_THN:O_WQ] = theta_w.astype(bf).reshape(-1)
    pack[O_WQ:O_TV] = weight.astype(bf).reshape(-1)
    pack[O_TV:O_EYE] = np.broadcast_to(t.astype(bf), (128, M)).reshape(-1)
    pack[O_EYE:O_RW] = np.eye(128, dtype=np.float32).astype(bf).reshape(-1)
    pack[O_RW:] = np.ascontiguousarray(
        rw.reshape(NT, 128).T).astype(bf).reshape(-1)
    return {
        "xnat": X.reshape(B * N, D).astype(bf),           # sharded over cores
        "pack": pack.reshape(1, PACK_LEN),
    }


# ---------------------------------------------------------------------------
# Cached-jit executor: trace/compile once per process, then each call is
# host->device of ~12 MB + one dispatch. Replicated inputs use P(None) so a
# single copy is shipped rather than 8.
# ---------------------------------------------------------------------------

_EXEC_CACHE = {}


def _get_exec():
    key = REPEAT
    if key in _EXEC_CACHE:
        return _EXEC_CACHE[key]
    import jax
    from jax.sharding import Mesh, PartitionSpec
    from jax.experimental.shard_map import shard_map
    from concourse import mybir as _mybir
    from concourse.bass2jax import (
        _bass_exec_p, install_neuronx_cc_hook, partition_id_tensor,
    )

    install_neuronx_cc_hook()
    nc = _get_nc()

    partition_name = (nc.partition_id_tensor.name
                      if nc.partition_id_tensor else None)
    in_names, out_names, out_avals, zero_outs = [], [], [], []
    for alloc in nc.m.functions[0].allocations:
        if not isinstance(alloc, _mybir.MemoryLocationSet):
            continue
        name = alloc.memorylocations[0].name
        if alloc.kind == "ExternalInput":
            if name == partition_name:
                continue
            in_names.append(name)
        elif alloc.kind == "ExternalOutput":
            out_names.append(name)
            shape = tuple(alloc.tensor_shape)
            dtype = _mybir.dt.np(alloc.dtype)
            out_avals.append(jax.core.ShapedArray(shape, dtype))
            zero_outs.append(np.zeros(shape, dtype))
    n_params = len(in_names)
    all_names = in_names + out_names
    if partition_name is not None:
        all_names = all_names + [partition_name]

    def _body(*args):
        operands = list(args)
        if partition_name is not None:
            operands.append(partition_id_tensor())
        outs = _bass_exec_p.bind(
            *operands,
            out_avals=tuple(out_avals),
            in_names=tuple(all_names),
            out_names=tuple(out_names),
            lowering_input_output_aliases=(),
            sim_require_finite=True,
            sim_require_nnan=True,
            nc=nc,
        )
        return tuple(outs)

    devices = jax.devices()[:N_CORES]
    mesh = Mesh(np.asarray(devices), ("core",))
    in_specs = tuple(
        PartitionSpec("core") if nm in _SHARDED else PartitionSpec()
        for nm in in_names
    ) + (PartitionSpec("core"),) * len(zero_outs)
    out_specs = (PartitionSpec("core"),) * len(out_names)
    fn = jax.jit(
        shard_map(_body, mesh=mesh,
                  in_specs=in_specs, out_specs=out_specs,
                  check_rep=False),
        keep_unused=True,
    )
    shardings = [jax.sharding.NamedSharding(mesh, sp) for sp in in_specs]
    res = (fn, in_names, out_names, zero_outs, shardings)
    _EXEC_CACHE[key] = res
    return res


def _run_jit(glob_inputs):
    import jax
    fn, in_names, out_names, zero_outs, shardings = _get_exec()
    args = [glob_inputs[nm] for nm in in_names]
    args += [np.zeros((N_CORES * z.shape[0], *z.shape[1:]), z.dtype)
             for z in zero_outs]
    # one batched H2D (per-array puts each pay an axon roundtrip)
    try:
        dargs = jax.device_put(args, shardings)
    except Exception:
        dargs = args
    outs = fn(*dargs)
    jax.block_until_ready(outs)
    return {nm: np.asarray(o) for nm, o in zip(out_names, outs)}


def _run_fallback(glob_inputs):
    """Per-core run via run_bass_kernel_spmd (native-NRT capable path)."""
    from concourse.bass_utils import run_bass_kernel_spmd
    nc = _get_nc()
    in_maps = []
    for c in range(N_CORES):
        m = {}
        for nm, arr in glob_inputs.items():
            if nm in _SHARDED:
                sh = arr.shape[0] // N_CORES
                m[nm] = arr[c * sh:(c + 1) * sh]
            else:
                m[nm] = arr
        in_maps.append(m)
    res = run_bass_kernel_spmd(nc, in_maps, list(range(N_CORES)))
    outs = res.results if hasattr(res, "results") else res
    return {"out": np.concatenate([o["out"] for o in outs], axis=0)}


def _assemble(out_concat):
    out_full = np.empty((B, P), dtype=np.float32)
    for c in range(N_CORES):
        o = out_concat[c * 128:(c + 1) * 128]
        out_full[c] = np.ascontiguousarray(o.T).reshape(P)
    return out_full / REPEAT


def kernel(X, theta_w, ref, weight):
    import time as _time

    glob_inputs = _host_precompute(X, theta_w, ref, weight)
    last_err = None
    for attempt in range(3):
        try:
            outs = _run_jit(glob_inputs)
            return _assemble(outs["out"])
        except Exception as e:  # transient transport errors (mesh desync)
            last_err = e
            _time.sleep(3)
    # final fallback: sanctioned spmd runner (works native or axon)
    try:
        outs = _run_fallback(glob_inputs)
        return _assemble(outs["out"])
    except Exception:
        raise last_err


# ---------------------------------------------------------------------------
# Benchmark path: cached jit + device-resident inputs, excludes host transfer.
# ---------------------------------------------------------------------------

def make_bench(X, theta_w, ref, weight):
    import jax

    fn, in_names, out_names, zero_outs, _shardings = _get_exec()
    glob_inputs = _host_precompute(X, theta_w, ref, weight)
    args = [glob_inputs[nm] for nm in in_names]
    args += [np.zeros((N_CORES * z.shape[0], *z.shape[1:]), z.dtype)
             for z in zero_outs]
    dev_in = [jax.device_put(a) for a in args]

    def run():
        outs = fn(*dev_in)
        jax.block_until_ready(outs)
        return outs

    def collect(outs):
        return _assemble(np.asarray(outs[0]))

    return run, collect


# revision 10
# speedup vs baseline: 1.1191x; 1.1191x over previous
"""Trainium2 Bass kernel for nn_FPSWE_40303973105696.

Computation (see problem reference): project X onto P directions, sort along
N, linearly interpolate N->M quantiles, subtract from ref, contract with
weight.

For these shapes (N=2048, M=1024) the quantile-interpolation gather is exactly
ind[m] = 2m, so the folded contraction weights are a simple even/odd
interleave of weight scaled by the interpolation fraction t[m]:

    out[b, p] = rw[p] - sum_n Xs[b, n, p] * W2[p, n]
    W2[p, 2m]   = weight[p, m] * (1 - t[m])
    W2[p, 2m+1] = weight[p, m] * t[m]
    rw[p]       = sum_m ref[m] * weight[p, m]

so W2 is built on-device from weight itself (no precomputed [P, N] upload).

Device kernel per core (data-parallel over B, core c handles batch c):
    1. transpose X[b] and theta_w via PE identity matmuls
    2. proj[p, n] = theta_w[p, :] @ X[b].T        (PE, bf16 -> fp32 PSUM)
    3. bitonic sort along n                       (DVE bf16, 66 stages)
    4. W2 rows from weight + t                    (GPSIMD, overlapped)
    5. acc[p] = sum_n Xs[p, n] * W2[p, n]         (GPSIMD mult + ACT accum)
    6. out[p] = rw[p] - acc[p]

All 8 row-chunks share ONE wide [128, 16384] tile pair, with columns in the
relabeled order of sortnet.py (col = 8*z(n) + chunk). The relabeling keeps
every compare-exchange operand's innermost contiguous run >= 8 unit-stride
elements, which is what the DVE's packed bf16 perf modes require; the
measured penalty for innermost runs of 1 (classic stride-1 bitonic stages)
is ~4x.
"""

import numpy as np

from concourse import bass, bacc, mybir
from concourse.ap import AP
from concourse.tile import TileContext

import sortnet

B, N, D, P, M = 8, 2048, 128, 1024, 1024
NT = P // 128          # 8 projection row-chunks of 128 partitions each
MM_CHUNK = 512         # matmul free-dim chunk (one PSUM bank)
N_CORES = 8

# sub-tile groups: (ci = chunks interleaved, global chunk ids, emu_frac =
# fraction of each std stage offloaded to the GPSIMD+ACT relu-emulated
# compare-exchange pipeline)
GROUPS_CFG = [
    (4, [0, 1, 2, 3], 0.0),
    (4, [4, 5, 6, 7], 0.0),
]

# debug knob: limit number of sort stages emitted (None = all)
STAGE_LIMIT = None
# benchmark knob: emit the whole kernel body this many times (timing only)
REPEAT = 1

FP = mybir.dt.float32
SD = mybir.dt.bfloat16  # sort dtype

_ALU = {"min": mybir.AluOpType.min, "max": mybir.AluOpType.max}

# input names that are per-core (sharded); everything else is replicated
_SHARDED = {"xnat"}

# packed replicated-input layout (bf16 elements): theta_w | weight |
# t-vector (replicated to 128 partitions) | identity | rw
O_THN = 0
O_WQ = O_THN + P * D
O_TV = O_WQ + P * M
O_EYE = O_TV + 128 * M
O_RW = O_EYE + 128 * 128
PACK_LEN = O_RW + 128 * NT


def _ap(tile_ap, dims, off):
    """Build an AP on a [128, W] tile from (col-dims, col-offset)."""
    base = tile_ap[:]
    w = base.ap[0][0]          # partition stride = row width
    return AP(base.tensor, int(base.offset) + off,
              [[w, 128]] + [[s, c] for s, c in dims])


def _build_kernel():
    nc = bacc.Bacc()

    xnat = nc.declare_dram_parameter("xnat", [N, D], SD, isOutput=False)   # X[b]
    pack = nc.declare_dram_parameter("pack", [1, PACK_LEN], SD, isOutput=False)
    out = nc.declare_dram_parameter("out", [128, NT], FP, isOutput=True)

    stages = sortnet.stages()
    if STAGE_LIMIT is not None:
        stages = stages[:STAGE_LIMIT]

    with TileContext(nc) as tc:
        with (
            tc.tile_pool(name="const", bufs=1) as const_pool,
            tc.tile_pool(name="xt", bufs=1) as xt_pool,
            tc.tile_pool(name="sa", bufs=1) as a_pool,
            tc.tile_pool(name="sb", bufs=1) as b_pool,
            tc.tile_pool(name="wq", bufs=2) as w_pool,
            tc.tile_pool(name="w2", bufs=1) as w2_pool,
            tc.tile_pool(name="prod", bufs=2) as prod_pool,
            tc.tile_pool(name="pst", bufs=2, space="PSUM") as pst_pool,
            tc.tile_pool(name="ps", bufs=2, space="PSUM") as psum_pool,
        ):
            eye_raw = const_pool.tile([128, 128], SD, tag="eyer")
            eye_sb = const_pool.tile([128, 128], SD, tag="eye")
            tv_sb = const_pool.tile([128, M], SD, tag="tv")
            tvb_sb = const_pool.tile([128, M], SD, tag="tvb")
            rw_sb = const_pool.tile([128, NT], SD, tag="rw")
            acc_sb = const_pool.tile([128, NT], FP, tag="acc")
            out_sb = const_pool.tile([128, NT], FP, tag="outsb")
            out_tmp = const_pool.tile([128, NT], FP, tag="outtmp")
            xn_sb = xt_pool.tile([128, N], SD, tag="xn")        # X[b] natural
            xt_sb = xt_pool.tile([D, N], SD, tag="xt")          # X[b].T
            thn_sb = xt_pool.tile([128, P], SD, tag="thn")      # theta_w natural
            tht_sb = xt_pool.tile([D, P], SD, tag="tht")        # theta_w.T

            nc.sync.dma_start(
                out=eye_raw[:],
                in_=pack[0:1, O_EYE:O_EYE + 128 * 128].rearrange(
                    "o (p q) -> (o p) q", q=128))
            nc.sync.dma_start(
                out=tv_sb[:],
                in_=pack[0:1, O_TV:O_TV + 128 * M].rearrange(
                    "o (p m) -> (o p) m", m=M))
            nc.sync.dma_start(
                out=rw_sb[:],
                in_=pack[0:1, O_RW:O_RW + 128 * NT].rearrange(
                    "o (p r) -> (o p) r", r=NT))
            # X[b] as [128, 16*128]: xn_sb[p, k*128+d] = X[k*128+p, d]
            nc.sync.dma_start(
                out=xn_sb.rearrange("p (k d) -> p k d", d=D),
                in_=xnat.rearrange("(k p) d -> p k d", p=128))
            # theta_w as [128, 8*128]: thn_sb[p, r*128+d] = theta_w[r*128+p, d]
            nc.sync.dma_start(
                out=thn_sb.rearrange("p (r d) -> p r d", d=D),
                in_=pack[0:1, O_THN:O_THN + P * D].rearrange(
                    "o (r p d) -> (o p) r d", p=128, d=D))
            # Bounce eye through ACT so Matmult (transpose) instructions never
            # carry two DMA-queue semaphore waits (walrus codegen limit).
            nc.scalar.copy(out=eye_sb[:], in_=eye_raw[:])
            # bounce tv through ACT so gpsimd W2-build ops carry at most one
            # DMA-queue semaphore wait
            nc.scalar.copy(out=tvb_sb[:], in_=tv_sb[:])

            # on-device transposes: X[b].T [D, N] and theta_w.T [D, P]
            for k in range(N // 128):
                ps = pst_pool.tile([128, 128], SD, tag="pst", name="pst")
                nc.tensor.transpose(
                    ps[:], xn_sb[:, k * 128:(k + 1) * 128], eye_sb[:])
                nc.scalar.copy(out=xt_sb[:, k * 128:(k + 1) * 128], in_=ps[:])
            for r in range(NT):
                ps = pst_pool.tile([128, 128], SD, tag="pst", name="pst")
                nc.tensor.transpose(
                    ps[:], thn_sb[:, r * 128:(r + 1) * 128], eye_sb[:])
                nc.scalar.copy(out=tht_sb[:, r * 128:(r + 1) * 128], in_=ps[:])

            # Per-group W2 (relabeled layout), built on GPSIMD
            ngr = len(GROUPS_CFG)
            nsets = 2 if REPEAT > 1 else 1
            w2_t, a_t, b_t = {}, {}, {}
            for g, (ci, gchunks, _ef) in enumerate(GROUPS_CFG):
                gw = ci * N
                w2_t[g] = w2_pool.tile([128, gw], SD, tag=f"w2f{g}", name=f"w2f{g}")
                for s in range(nsets):
                    a_t[(g, s)] = a_pool.tile([128, gw], SD,
                                              tag=f"aw{g}_{s}",
                                              name=f"aw{g}_{s}")
                    b_t[(g, s)] = b_pool.tile([128, gw], SD,
                                              tag=f"bw{g}_{s}",
                                              name=f"bw{g}_{s}")
                for rl, gid in enumerate(gchunks):
                    w_sb = w_pool.tile([128, M], SD, tag="wq", name="wq")
                    nc.sync.dma_start(
                        out=w_sb[:],
                        in_=pack[0:1,
                                 O_WQ + gid * 128 * M:
                                 O_WQ + (gid + 1) * 128 * M].rearrange(
                            "o (p m) -> (o p) m", m=M))
                    # odd (e=1): W2 = w * t ; even (e=0): W2 = w - odd
                    for kind, od, oo, i0d, i0o, i1d, i1o in \
                            sortnet.w2_ops(rl, ci):
                        if kind == "mult":
                            nc.gpsimd.tensor_mul(_ap(w2_t[g], od, oo),
                                                 _ap(w_sb, i0d, i0o),
                                                 _ap(tvb_sb, i1d, i1o))
                        else:
                            nc.gpsimd.tensor_sub(_ap(w2_t[g], od, oo),
                                                 _ap(w_sb, i0d, i0o),
                                                 _ap(w2_t[g], i1d, i1o))

            # emu scratch (difference + relu tiles), only when used
            if any(ef > 0.0 for _, _, ef in GROUPS_CFG):
                d_t = a_pool.tile([128, 4096], SD, tag="demul", name="demul")
                r_t = a_pool.tile([128, 4096], SD, tag="remul", name="remul")

            def _dense(counts):
                dims = []
                prod = 1
                for c in reversed(counts):
                    dims.append((prod, c))
                    prod *= c
                dims.reverse()
                return dims

            def _emit_emu(src, dst, minop, maxop, emu_frac):
                """Split a std-stage op pair: DVE head slice + GPSIMD/ACT
                relu-emulated tail slice of dim 0."""
                _, od0, oo0, i0d, i0o, i1d, i1o = minop
                _, od1, oo1, _, _, _, _ = maxop
                s0, c0 = od0[0]
                k = int(round((1.0 - emu_frac) * c0))
                if s0 == 1:
                    k = max(0, min(c0, (k // 2) * 2))
                k = max(0, min(c0, k))

                def part(dims, off, lo, hi):
                    nd = [(dims[0][0], hi - lo)] + list(dims[1:])
                    return nd, off + dims[0][0] * lo

                if k > 0:
                    for alu, od, oo, j0d, j0o, j1d, j1o in (minop, maxop):
                        pd, po = part(od, oo, 0, k)
                        p0d, p0o = part(j0d, j0o, 0, k)
                        p1d, p1o = part(j1d, j1o, 0, k)
                        nc.vector.tensor_tensor(
                            _ap(dst, pd, po), _ap(src, p0d, p0o),
                            _ap(src, p1d, p1o), op=_ALU[alu])
                if k < c0:
                    lod, loo = part(i0d, i0o, k, c0)
                    hid, hio = part(i1d, i1o, k, c0)
                    om_d, om_o = part(od0, oo0, k, c0)
                    ox_d, ox_o = part(od1, oo1, k, c0)
                    counts = [c for _, c in lod]
                    dd = _dense(counts)
                    nc.gpsimd.tensor_sub(_ap(d_t, dd, 0),
                                         _ap(src, lod, loo),
                                         _ap(src, hid, hio))
                    nc.scalar.activation(_ap(r_t, dd, 0), _ap(d_t, dd, 0),
                                         mybir.ActivationFunctionType.Relu)
                    nc.gpsimd.tensor_sub(_ap(dst, om_d, om_o),
                                         _ap(src, lod, loo),
                                         _ap(r_t, dd, 0))
                    nc.gpsimd.tensor_add(_ap(dst, ox_d, ox_o),
                                         _ap(src, hid, hio),
                                         _ap(r_t, dd, 0))

            def emit_body(rep_i):
                ts = rep_i % nsets
                # ---- phase A: projection matmuls, scattered into the
                # relabeled group layouts (group-major for overlap) ----
                for g, (ci, gchunks, _ef) in enumerate(GROUPS_CFG):
                    for rl, gid in enumerate(gchunks):
                        for c in range(N // MM_CHUNK):
                            ps = psum_pool.tile([128, MM_CHUNK], FP,
                                                tag="ps", name="ps")
                            nc.tensor.matmul(
                                ps[:],
                                lhsT=tht_sb[:, gid * 128:(gid + 1) * 128],
                                rhs=xt_sb[:, c * MM_CHUNK:(c + 1) * MM_CHUNK],
                                start=True, stop=True,
                            )
                            in_dims, out_dims, out_off = \
                                sortnet.proj_copy_aps(rl, c, ci)
                            nc.scalar.copy(
                                out=_ap(a_t[(g, ts)], out_dims, out_off),
                                in_=_ap(ps, in_dims, 0))

                # ---- phase B: sort (groups round-robin per stage) ----
                curs = {g: a_t[(g, ts)] for g in range(ngr)}
                oths = {g: b_t[(g, ts)] for g in range(ngr)}
                for kind, val in stages:
                    for g, (ci, gchunks, ef) in enumerate(GROUPS_CFG):
                        ops = sortnet.stage_ops(kind, val, ci)
                        if ef > 0.0 and kind == "std" and len(ops) == 2:
                            _emit_emu(curs[g], oths[g], ops[0], ops[1], ef)
                            continue
                        for alu, od, oo, i0d, i0o, i1d, i1o in ops:
                            nc.vector.tensor_tensor(
                                _ap(oths[g], od, oo),
                                _ap(curs[g], i0d, i0o),
                                _ap(curs[g], i1d, i1o),
                                op=_ALU[alu])
                    curs, oths = oths, curs

                # ---- phase C: weighted reduction per row-chunk ----
                for g, (ci, gchunks, _ef) in enumerate(GROUPS_CFG):
                    for rl, gid in enumerate(gchunks):
                        rd, ro = sortnet.reduce_ap(rl, ci)
                        prod = prod_pool.tile([128, N], FP, tag="prod",
                                              name="prod")
                        nc.gpsimd.tensor_mul(prod[:],
                                             _ap(curs[g], rd, ro),
                                             _ap(w2_t[g], rd, ro))
                        nc.scalar.activation(
                            _ap(curs[g], rd, ro), prod[:],
                            mybir.ActivationFunctionType.Copy,
                            accum_out=acc_sb[:, gid:gid + 1])

                # accumulate across repeat bodies so none is dead code;
                # the final output is REPEAT * (rw - acc), divided on host
                if rep_i == 0:
                    nc.vector.tensor_sub(out_sb[:], rw_sb[:], acc_sb[:])
                else:
                    nc.vector.tensor_sub(out_tmp[:], rw_sb[:], acc_sb[:])
                    nc.vector.tensor_add(out_sb[:], out_sb[:], out_tmp[:])

            for _rep in range(REPEAT):
                emit_body(_rep)
            nc.sync.dma_start(out=out[:], in_=out_sb[:])

    return nc


_NC_CACHE = None


def _get_nc():
    global _NC_CACHE
    if _NC_CACHE is None:
        nc = _build_kernel()
        nc.finalize()   # Bacc: runs wait-splitting + register allocation
        _NC_CACHE = nc
    return _NC_CACHE


def _host_precompute(X, theta_w, ref, weight):
    """Global (all-core) input arrays, keyed by dram parameter name."""
    X = np.ascontiguousarray(np.asarray(X, dtype=np.float32))
    theta_w = np.ascontiguousarray(np.asarray(theta_w, dtype=np.float32))
    ref = np.asarray(ref, dtype=np.float32)
    weight = np.ascontiguousarray(np.asarray(weight, dtype=np.float32))

    x1d = np.linspace(0.0, 1.0, N + 2, dtype=np.float32)[1:-1]
    xnew = np.linspace(0.0, 1.0, M + 2, dtype=np.float32)[1:-1]
    ind = 2 * np.arange(M)      # == clip(searchsorted(x1d, xnew) - 1, 0, N-2)
    eps = np.float32(np.finfo(np.float32).eps)
    dx = x1d[1:] - x1d[:-1]
    t = ((xnew - x1d[ind]) / (eps + dx[ind])).astype(np.float32)

    rw = (weight.astype(np.float64) @ ref.astype(np.float64)).astype(np.float32)

    import ml_dtypes
    bf = ml_dtypes.bfloat16
    pack = np.empty(PACK_LEN, dtype=bf)
    pack[O_THN:O_WQ] = theta_w.astype(bf).reshape(-1)
    pack[O_WQ:O_TV] = weight.astype(bf).reshape(-1)
    pack[O_TV:O_EYE] = np.broadcast_to(t.astype(bf), (128, M)).reshape(-1)
    pack[O_EYE:O_RW] = np.eye(128, dtype=np.float32).astype(bf).reshape(-1)
    pack[O_RW:] = np.ascontiguousarray(
        rw.reshape(NT, 128).T).astype(bf).reshape(-1)
    return {
        "xnat": X.reshape(B * N, D).astype(bf),           # sharded over cores
        "pack": pack.reshape(1, PACK_LEN),
    }


# ---------------------------------------------------------------------------
# Cached-jit executor: trace/compile once per process, then each call is
# host->device of ~12 MB + one dispatch. Replicated inputs use P(None) so a
# single copy is shipped rather than 8.
# ---------------------------------------------------------------------------

_EXEC_CACHE = {}


def _get_exec():
    key = REPEAT
    if key in _EXEC_CACHE:
        return _EXEC_CACHE[key]
    import jax
    from jax.sharding import Mesh, PartitionSpec
    from jax.experimental.shard_map import shard_map
    from concourse import mybir as _mybir
    from concourse.bass2jax import (
        _bass_exec_p, install_neuronx_cc_hook, partition_id_tensor,
    )

    install_neuronx_cc_hook()
    nc = _get_nc()

    partition_name = (nc.partition_id_tensor.name
                      if nc.partition_id_tensor else None)
    in_names, out_names, out_avals, zero_outs = [], [], [], []
    for alloc in nc.m.functions[0].allocations:
        if not isinstance(alloc, _mybir.MemoryLocationSet):
            continue
        name = alloc.memorylocations[0].name
        if alloc.kind == "ExternalInput":
            if name == partition_name:
                continue
            in_names.append(name)
        elif alloc.kind == "ExternalOutput":
            out_names.append(name)
            shape = tuple(alloc.tensor_shape)
            dtype = _mybir.dt.np(alloc.dtype)
            out_avals.append(jax.core.ShapedArray(shape, dtype))
            zero_outs.append(np.zeros(shape, dtype))
    n_params = len(in_names)
    all_names = in_names + out_names
    if partition_name is not None:
        all_names = all_names + [partition_name]

    def _body(*args):
        operands = list(args)
        if partition_name is not None:
            operands.append(partition_id_tensor())
        outs = _bass_exec_p.bind(
            *operands,
            out_avals=tuple(out_avals),
            in_names=tuple(all_names),
            out_names=tuple(out_names),
            lowering_input_output_aliases=(),
            sim_require_finite=True,
            sim_require_nnan=True,
            nc=nc,
        )
        return tuple(outs)

    devices = jax.devices()[:N_CORES]
    mesh = Mesh(np.asarray(devices), ("core",))
    in_specs = tuple(
        PartitionSpec("core") if nm in _SHARDED else PartitionSpec()
        for nm in in_names
    ) + (PartitionSpec("core"),) * len(zero_outs)
    out_specs = (PartitionSpec("core"),) * len(out_names)
    fn = jax.jit(
        shard_map(_body, mesh=mesh,
                  in_specs=in_specs, out_specs=out_specs,
                  check_rep=False),
        keep_unused=True,
    )
    shardings = [jax.sharding.NamedSharding(mesh, sp) for sp in in_specs]
    res = (fn, in_names, out_names, zero_outs, shardings)
    _EXEC_CACHE[key] = res
    return res


def _run_jit(glob_inputs):
    import jax
    fn, in_names, out_names, zero_outs, shardings = _get_exec()
    args = [glob_inputs[nm] for nm in in_names]
    args += [np.zeros((N_CORES * z.shape[0], *z.shape[1:]), z.dtype)
             for z in zero_outs]
    # one batched H2D (per-array puts each pay an axon roundtrip)
    try:
        dargs = jax.device_put(args, shardings)
    except Exception:
        dargs = args
    outs = fn(*dargs)
    jax.block_until_ready(outs)
    return {nm: np.asarray(o) for nm, o in zip(out_names, outs)}


def _run_fallback(glob_inputs):
    """Per-core run via run_bass_kernel_spmd (native-NRT capable path)."""
    from concourse.bass_utils import run_bass_kernel_spmd
    nc = _get_nc()
    in_maps = []
    for c in range(N_CORES):
        m = {}
        for nm, arr in glob_inputs.items():
            if nm in _SHARDED:
                sh = arr.shape[0] // N_CORES
                m[nm] = arr[c * sh:(c + 1) * sh]
            else:
                m[nm] = arr
        in_maps.append(m)
    res = run_bass_kernel_spmd(nc, in_maps, list(range(N_CORES)))
    outs = res.results if hasattr(res, "results") else res
    return {"out": np.concatenate([o["out"] for o in outs], axis=0)}


def _assemble(out_concat):
    out_full = np.empty((B, P), dtype=np.float32)
    for c in range(N_CORES):
        o = out_concat[c * 128:(c + 1) * 128]
        out_full[c] = np.ascontiguousarray(o.T).reshape(P)
    return out_full / REPEAT


def kernel(X, theta_w, ref, weight):
    import time as _time

    glob_inputs = _host_precompute(X, theta_w, ref, weight)
    last_err = None
    for attempt in range(3):
        try:
            outs = _run_jit(glob_inputs)
            return _assemble(outs["out"])
        except Exception as e:  # transient transport errors (mesh desync)
            last_err = e
            _time.sleep(3)
    # final fallback: sanctioned spmd runner (works native or axon)
    try:
        outs = _run_fallback(glob_inputs)
        return _assemble(outs["out"])
    except Exception:
        raise last_err


# ---------------------------------------------------------------------------
# Benchmark path: cached jit + device-resident inputs, excludes host transfer.
# ---------------------------------------------------------------------------

def make_bench(X, theta_w, ref, weight):
    import jax

    fn, in_names, out_names, zero_outs, _shardings = _get_exec()
    glob_inputs = _host_precompute(X, theta_w, ref, weight)
    args = [glob_inputs[nm] for nm in in_names]
    args += [np.zeros((N_CORES * z.shape[0], *z.shape[1:]), z.dtype)
             for z in zero_outs]
    dev_in = [jax.device_put(a) for a in args]

    def run():
        outs = fn(*dev_in)
        jax.block_until_ready(outs)
        return outs

    def collect(outs):
        return _assemble(np.asarray(outs[0]))

    return run, collect
